# revision 26
# baseline (speedup 1.0000x reference)
"""Bit-exact Trainium2 Bass kernel for the DiehlCook SNN reference.

Data-parallel over batch (128 -> 16 per core x 8 cores). Reproduces the
neuronxcc reference lowering bit-for-bit:
  - matmuls fp32 on PE, K tiled in 128-groups ascending, PSUM-accumulated in
    order (spikes are 0/1 so products are exact; operand swap verified bit-safe)
  - LIF elementwise as discrete IEEE f32 DVE/GPSIMD ops matching the HLO
    dataflow (sign-flipped decay carry nb = fl(u*0.01) - u is IEEE-exact)
  - inh_spikes == exc_spikes delayed one step (exact property), done on host.

Phase-1 (I_in[t] = x_t @ w_in, all t) is interleaved into the phase-2
recurrence loop so its PE/ACT work fills the recurrence's engine gaps.
"""
import numpy as np

T = 500
PAIRS = T // 2
B_SH = 16          # batch per core
NCORES = 8
G0 = 7             # k-groups for dot0 (784 -> 6*128 + 112)
G1 = 4             # k-groups for dot1 (400 padded to 512; pad rows stay 0)
NG = 4             # n-groups (400 -> 4*128; lanes >=400 stay identically 0)
CH_T = 32          # timesteps per phase-1 chunk (32*16 = 512 moving columns)
NCHUNK = (T + CH_T - 1) // CH_T
LOOKAHEAD = 1      # chunks emitted ahead of the pairs that consume them
INTERLEAVE = True  # merge phase-1 into the recurrence loop
USE_STT = True     # fused scalar_tensor_tensor decay (nb carry)
P1_COPY_ACT = False
S_ON_DVE = True    # is_ge + psum copy on DVE (fewer cross-engine hops)

_CACHE = {}


def _build():
    import concourse.bass as bass
    import concourse.tile as tile
    from concourse import bacc, mybir

    F32 = mybir.dt.float32
    I32 = mybir.dt.int32
    OP = mybir.AluOpType
    nchunk = NCHUNK
    pairs = PAIRS

    nc = bacc.Bacc("TRN2", target_bir_lowering=False, debug=False)

    xt = nc.dram_tensor("xt", [G0, 128, T, B_SH], F32, kind="ExternalInput")
    w_in = nc.dram_tensor("w_in", [G0, 128, 512], F32, kind="ExternalInput")
    w_ie = nc.dram_tensor("w_ie", [G1, 128, 512], F32, kind="ExternalInput")
    ident = nc.dram_tensor("ident", [32, 32], F32, kind="ExternalInput")
    s_out = nc.dram_tensor("s_out", [pairs, 128, NG, 2, B_SH], F32, kind="ExternalOutput")

    with tile.TileContext(nc) as tc:
        with (
            tc.tile_pool(name="const", bufs=1) as constp,
            tc.tile_pool(name="iinp", bufs=1) as iinp,
            tc.tile_pool(name="xtp", bufs=2) as xtp,
            tc.tile_pool(name="ps1", bufs=2, space="PSUM") as ps1,
            tc.tile_pool(name="bmps", bufs=2, space="PSUM") as bmps,
            tc.tile_pool(name="nmps", bufs=2, space="PSUM") as nmps,
            tc.tile_pool(name="bmsb", bufs=3) as bmsb,
            tc.tile_pool(name="s1p", bufs=4) as s1p,
            tc.tile_pool(name="sp", bufs=4) as sp,
            tc.tile_pool(name="uv", bufs=4) as uvp,
        ):
            wi_sb = constp.tile([128, G0, 512], F32)
            for g in range(G0):
                nc.sync.dma_start(wi_sb[:, g, :], w_in[g])
            wie_sb = constp.tile([128, G1, 512], F32)
            for g in range(G1):
                nc.sync.dma_start(wie_sb[:, g, :], w_ie[g])
            ident_sb = constp.tile([32, 32], F32)
            nc.sync.dma_start(ident_sb[:], ident[:])

            szero = constp.tile([128, NG, 2, B_SH], F32)
            nc.gpsimd.memset(szero[:], 0.0)
            nbzero = constp.tile([128, NG, B_SH], F32)
            nc.gpsimd.memset(nbzero[:], 0.0)

            iin_tiles = [None] * (pairs // 2)

            def emit_chunk(ch):
                """dot0 for timesteps [ch*CH_T, ...): fills iin_tiles pairs."""
                t0 = ch * CH_T
                t1 = min(T, t0 + CH_T)
                ccols = (t1 - t0) * B_SH
                npair = (t1 - t0) // 2
                xts = xtp.tile([128, G0, CH_T * B_SH], F32, name=f"xts{ch}", tag="xts")
                for g in range(G0):
                    kk = 128 if g < G0 - 1 else 112
                    nc.sync.dma_start(xts[0:kk, g, 0:ccols], xt[g, 0:kk, t0:t1, :])
                for q in range(npair // 2):
                    iin_tiles[t0 // 4 + q] = iinp.tile(
                        [128, NG, 2, 2, B_SH], F32,
                        name=f"iin{t0 // 4 + q}", tag=f"iin{t0 // 4 + q}")
                for ng in range(NG):
                    ps = ps1.tile([128, 512], F32, name=f"ps{ch}_{ng}", tag="p1")
                    for g in range(G0):
                        kk = 128 if g < G0 - 1 else 112
                        nc.tensor.matmul(
                            ps[:, 0:ccols],
                            wi_sb[0:kk, g, ng * 128:(ng + 1) * 128],
                            xts[0:kk, g, 0:ccols],
                            start=(g == 0),
                            stop=(g == G0 - 1),
                        )
                    for q in range(npair // 2):
                        nc.vector.tensor_copy(
                            iin_tiles[t0 // 4 + q][:, ng],
                            ps[:, 64 * q:64 * q + 64].rearrange(
                                "n (p s b) -> n p s b", p=2, s=2, b=B_SH),
                        )

            if INTERLEAVE:
                for ch in range(min(LOOKAHEAD, nchunk)):
                    emit_chunk(ch)
            else:
                for ch in range(nchunk):
                    emit_chunk(ch)

            s_pair_prev = szero
            nb_prev = nbzero      # nb = fl(u*0.01) - u of prev step (spiked lanes unused)
            s_prev = szero[:, :, 0, :]

            for i in range(pairs):
                if INTERLEAVE and i % (CH_T // 2) == 0:
                    ch_need = i // (CH_T // 2) + LOOKAHEAD
                    if ch_need < nchunk:
                        emit_chunk(ch_need)

                psb = bmps.tile([32, 400], F32, name=f"psb{i}", tag="bm")
                for g in range(G1):
                    nc.tensor.matmul(
                        psb[:],
                        s_pair_prev[:, g],
                        wie_sb[:, g, 0:400],
                        start=(g == 0),
                        stop=(g == G1 - 1),
                    )
                bm = bmsb.tile([32, 400], F32, name=f"bm{i}", tag="bmsb")
                if S_ON_DVE:
                    nc.vector.tensor_copy(bm[:], psb[:])
                else:
                    nc.scalar.copy(bm[:], psb[:])

                nm = nmps.tile([128, NG, 32], F32, name=f"nm{i}", tag="nm")
                nc.vector.memset(nm[:, 3, :], 0.0)
                for c in range(NG):
                    cw = 128 if c < 3 else 16
                    nc.tensor.transpose(
                        nm[0:cw, c, :],
                        bm[:, 128 * c:128 * c + cw],
                        ident_sb[:],
                    )

                s1 = s1p.tile([128, NG, 2, B_SH], F32, name=f"s1_{i}", tag="s1")
                nc.vector.tensor_tensor(
                    s1[:],
                    iin_tiles[i // 2][:, :, i % 2],
                    nm[:].rearrange("n c (s b) -> n c s b", s=2, b=B_SH),
                    OP.subtract,
                )

                s_pair = sp.tile([128, NG, 2, B_SH], F32, name=f"sp{i}", tag="spair")

                # step A (t = 2i): u = fl(s1 - nb_prev); spiked(prev) lanes -> s1
                s1A = s1[:, :, 0, :]
                uA = uvp.tile([128, NG, B_SH], F32, name=f"uA{i}", tag="u")
                nc.vector.tensor_tensor(uA[:], s1A, nb_prev[:], OP.subtract)
                nc.vector.copy_predicated(uA[:], s_prev.bitcast(I32), s1A)
                sA = s_pair[:, :, 0, :]
                if S_ON_DVE:
                    nc.vector.tensor_scalar(sA, uA[:], 1.0, None, OP.is_ge)
                else:
                    nc.gpsimd.tensor_scalar(sA, uA[:], 1.0, None, OP.is_ge)
                nbA = uvp.tile([128, NG, B_SH], F32, name=f"nbA{i}", tag="nb")
                if USE_STT:
                    nc.vector.scalar_tensor_tensor(
                        nbA[:], uA[:], 0.01, uA[:], OP.mult, OP.subtract)
                else:
                    aA = uvp.tile([128, NG, B_SH], F32, name=f"aA{i}", tag="a")
                    nc.vector.tensor_scalar_mul(aA[:], uA[:], 0.01)
                    nc.vector.tensor_tensor(nbA[:], aA[:], uA[:], OP.subtract)

                # step B (t = 2i+1)
                s1B = s1[:, :, 1, :]
                uB = uvp.tile([128, NG, B_SH], F32, name=f"uB{i}", tag="u")
                nc.vector.tensor_tensor(uB[:], s1B, nbA[:], OP.subtract)
                nc.vector.copy_predicated(uB[:], sA.bitcast(I32), s1B)
                sB = s_pair[:, :, 1, :]
                if S_ON_DVE:
                    nc.vector.tensor_scalar(sB, uB[:], 1.0, None, OP.is_ge)
                else:
                    nc.gpsimd.tensor_scalar(sB, uB[:], 1.0, None, OP.is_ge)
                nbB = uvp.tile([128, NG, B_SH], F32, name=f"nbB{i}", tag="nb")
                if USE_STT:
                    nc.vector.scalar_tensor_tensor(
                        nbB[:], uB[:], 0.01, uB[:], OP.mult, OP.subtract)
                else:
                    aB = uvp.tile([128, NG, B_SH], F32, name=f"aB{i}", tag="a")
                    nc.vector.tensor_scalar_mul(aB[:], uB[:], 0.01)
                    nc.vector.tensor_tensor(nbB[:], aB[:], uB[:], OP.subtract)

                nc.sync.dma_start(s_out[i], s_pair[:])

                s_pair_prev = s_pair
                nb_prev = nbB
                s_prev = sB

    nc.compile()
    return nc


def _prep_inputs(input_spikes, w_input_exc, w_inh_exc):
    x = np.ascontiguousarray(input_spikes, dtype=np.float32)     # (128, 784, 500)
    w_in = np.maximum(np.asarray(w_input_exc, dtype=np.float32), np.float32(0.0))
    mask = (np.float32(1.0) - np.eye(400, dtype=np.float32))
    w_ie = (np.maximum(np.asarray(w_inh_exc, dtype=np.float32),
                       np.float32(0.0)) * mask).astype(np.float32)

    w_in_pad = np.zeros((G0 * 128, 512), np.float32)
    w_in_pad[:784, :400] = w_in
    w_ie_pad = np.zeros((G1 * 128, 512), np.float32)
    w_ie_pad[:400, :400] = w_ie
    ident = np.eye(32, dtype=np.float32)

    in_maps = []
    for c in range(NCORES):
        xc = x[c * B_SH:(c + 1) * B_SH]                  # (16, 784, 500)
        xtc = np.zeros((G0 * 128, T, B_SH), np.float32)
        xtc[:784] = xc.transpose(1, 2, 0)                 # (784, 500, 16)
        in_maps.append({
            "xt": np.ascontiguousarray(xtc.reshape(G0, 128, T, B_SH)),
            "w_in": w_in_pad.reshape(G0, 128, 512),
            "w_ie": w_ie_pad.reshape(G1, 128, 512),
            "ident": ident,
        })
    return in_maps


def kernel(input_spikes, w_input_exc, w_inh_exc):
    from concourse.bass_utils import run_bass_kernel_spmd

    if "nc" not in _CACHE:
        _CACHE["nc"] = _build()
    nc = _CACHE["nc"]

    in_maps = _prep_inputs(input_spikes, w_input_exc, w_inh_exc)
    res = run_bass_kernel_spmd(nc, in_maps, core_ids=list(range(NCORES)))

    exc = np.empty((NCORES * B_SH, 400, T), np.float32)
    for c in range(NCORES):
        so = res.results[c]["s_out"]                      # (250, 128, 4, 2, 16)
        e = so.transpose(4, 2, 1, 0, 3).reshape(B_SH, NG * 128, T)
        exc[c * B_SH:(c + 1) * B_SH] = e[:, :400, :]
    inh = np.zeros_like(exc)
    inh[:, :, 1:] = exc[:, :, :-1]
    return exc, inh


# revision 27
# speedup vs baseline: 1.0435x; 1.0435x over previous
"""Bit-exact Trainium2 Bass kernel for the DiehlCook SNN reference.

Data-parallel over batch (128 -> 16 per core x 8 cores). Reproduces the
neuronxcc reference lowering bit-for-bit:
  - matmuls fp32 on PE, K tiled in 128-groups ascending, PSUM-accumulated in
    order (spikes are 0/1 so products are exact; operand swap verified bit-safe)
  - LIF elementwise as discrete IEEE f32 DVE/GPSIMD ops matching the HLO
    dataflow (sign-flipped decay carry nb = fl(u*0.01) - u is IEEE-exact)
  - inh_spikes == exc_spikes delayed one step (exact property), done on host.

Phase-1 (I_in[t] = x_t @ w_in, all t) is interleaved into the phase-2
recurrence loop so its PE/ACT work fills the recurrence's engine gaps.
"""
import numpy as np

T = 500
PAIRS = T // 2
B_SH = 16          # batch per core
NCORES = 8
G0 = 7             # k-groups for dot0 (784 -> 6*128 + 112)
G1 = 4             # k-groups for dot1 (400 padded to 512; pad rows stay 0)
NG = 4             # n-groups (400 -> 4*128; lanes >=400 stay identically 0)
CH_T = 16          # timesteps per phase-1 chunk (16*16 = 256 moving columns)
NCHUNK = (T + CH_T - 1) // CH_T
LOOKAHEAD = 1      # chunks emitted ahead of the pairs that consume them
INTERLEAVE = True  # merge phase-1 into the recurrence loop
USE_STT = True     # fused scalar_tensor_tensor decay (nb carry)
P1_COPY_ACT = False
S_ON_DVE = True    # is_ge + psum copy on DVE (fewer cross-engine hops)

_CACHE = {}


def _build():
    import concourse.bass as bass
    import concourse.tile as tile
    from concourse import bacc, mybir

    F32 = mybir.dt.float32
    I32 = mybir.dt.int32
    OP = mybir.AluOpType
    nchunk = NCHUNK
    pairs = PAIRS

    nc = bacc.Bacc("TRN2", target_bir_lowering=False, debug=False)

    xt = nc.dram_tensor("xt", [G0, 128, T, B_SH], F32, kind="ExternalInput")
    w_in = nc.dram_tensor("w_in", [G0, 128, 512], F32, kind="ExternalInput")
    w_ie = nc.dram_tensor("w_ie", [G1, 128, 512], F32, kind="ExternalInput")
    ident = nc.dram_tensor("ident", [32, 32], F32, kind="ExternalInput")
    s_out = nc.dram_tensor("s_out", [pairs, 128, NG, 2, B_SH], F32, kind="ExternalOutput")

    with tile.TileContext(nc) as tc:
        with (
            tc.tile_pool(name="const", bufs=1) as constp,
            tc.tile_pool(name="iinp", bufs=1) as iinp,
            tc.tile_pool(name="xtp", bufs=2) as xtp,
            tc.tile_pool(name="ps1", bufs=2, space="PSUM") as ps1,
            tc.tile_pool(name="bmps", bufs=2, space="PSUM") as bmps,
            tc.tile_pool(name="nmps", bufs=2, space="PSUM") as nmps,
            tc.tile_pool(name="bmsb", bufs=3) as bmsb,
            tc.tile_pool(name="s1p", bufs=4) as s1p,
            tc.tile_pool(name="sp", bufs=4) as sp,
            tc.tile_pool(name="uv", bufs=4) as uvp,
        ):
            wi_sb = constp.tile([128, G0, 512], F32)
            for g in range(G0):
                nc.sync.dma_start(wi_sb[:, g, :], w_in[g])
            wie_sb = constp.tile([128, G1, 512], F32)
            for g in range(G1):
                nc.sync.dma_start(wie_sb[:, g, :], w_ie[g])
            ident_sb = constp.tile([32, 32], F32)
            nc.sync.dma_start(ident_sb[:], ident[:])

            szero = constp.tile([128, NG, 2, B_SH], F32)
            nc.gpsimd.memset(szero[:], 0.0)
            nbzero = constp.tile([128, NG, B_SH], F32)
            nc.gpsimd.memset(nbzero[:], 0.0)

            iin_tiles = [None] * (pairs // 2)

            def emit_chunk(ch):
                """dot0 for timesteps [ch*CH_T, ...): fills iin_tiles pairs."""
                t0 = ch * CH_T
                t1 = min(T, t0 + CH_T)
                ccols = (t1 - t0) * B_SH
                npair = (t1 - t0) // 2
                xts = xtp.tile([128, G0, CH_T * B_SH], F32, name=f"xts{ch}", tag="xts")
                for g in range(G0):
                    kk = 128 if g < G0 - 1 else 112
                    nc.sync.dma_start(xts[0:kk, g, 0:ccols], xt[g, 0:kk, t0:t1, :])
                for q in range(npair // 2):
                    iin_tiles[t0 // 4 + q] = iinp.tile(
                        [128, NG, 2, 2, B_SH], F32,
                        name=f"iin{t0 // 4 + q}", tag=f"iin{t0 // 4 + q}")
                for ng in range(NG):
                    ps = ps1.tile([128, 512], F32, name=f"ps{ch}_{ng}", tag="p1")
                    for g in range(G0):
                        kk = 128 if g < G0 - 1 else 112
                        nc.tensor.matmul(
                            ps[:, 0:ccols],
                            wi_sb[0:kk, g, ng * 128:(ng + 1) * 128],
                            xts[0:kk, g, 0:ccols],
                            start=(g == 0),
                            stop=(g == G0 - 1),
                        )
                    for q in range(npair // 2):
                        nc.vector.tensor_copy(
                            iin_tiles[t0 // 4 + q][:, ng],
                            ps[:, 64 * q:64 * q + 64].rearrange(
                                "n (p s b) -> n p s b", p=2, s=2, b=B_SH),
                        )

            if INTERLEAVE:
                for ch in range(min(LOOKAHEAD, nchunk)):
                    emit_chunk(ch)
            else:
                for ch in range(nchunk):
                    emit_chunk(ch)

            s_pair_prev = szero
            nb_prev = nbzero      # nb = fl(u*0.01) - u of prev step (spiked lanes unused)
            s_prev = szero[:, :, 0, :]

            for i in range(pairs):
                if INTERLEAVE and i % (CH_T // 2) == 0:
                    ch_need = i // (CH_T // 2) + LOOKAHEAD
                    if ch_need < nchunk:
                        emit_chunk(ch_need)

                psb = bmps.tile([32, 400], F32, name=f"psb{i}", tag="bm")
                for g in range(G1):
                    nc.tensor.matmul(
                        psb[:],
                        s_pair_prev[:, g],
                        wie_sb[:, g, 0:400],
                        start=(g == 0),
                        stop=(g == G1 - 1),
                    )
                bm = bmsb.tile([32, 400], F32, name=f"bm{i}", tag="bmsb")
                if S_ON_DVE:
                    nc.vector.tensor_copy(bm[:], psb[:])
                else:
                    nc.scalar.copy(bm[:], psb[:])

                nm = nmps.tile([128, NG, 32], F32, name=f"nm{i}", tag="nm")
                nc.vector.memset(nm[:, 3, :], 0.0)
                for c in range(NG):
                    cw = 128 if c < 3 else 16
                    nc.tensor.transpose(
                        nm[0:cw, c, :],
                        bm[:, 128 * c:128 * c + cw],
                        ident_sb[:],
                    )

                s1 = s1p.tile([128, NG, 2, B_SH], F32, name=f"s1_{i}", tag="s1")
                nc.vector.tensor_tensor(
                    s1[:],
                    iin_tiles[i // 2][:, :, i % 2],
                    nm[:].rearrange("n c (s b) -> n c s b", s=2, b=B_SH),
                    OP.subtract,
                )

                s_pair = sp.tile([128, NG, 2, B_SH], F32, name=f"sp{i}", tag="spair")

                # step A (t = 2i): u = fl(s1 - nb_prev); spiked(prev) lanes -> s1
                s1A = s1[:, :, 0, :]
                uA = uvp.tile([128, NG, B_SH], F32, name=f"uA{i}", tag="u")
                nc.vector.tensor_tensor(uA[:], s1A, nb_prev[:], OP.subtract)
                nc.vector.copy_predicated(uA[:], s_prev.bitcast(I32), s1A)
                sA = s_pair[:, :, 0, :]
                if S_ON_DVE:
                    nc.vector.tensor_scalar(sA, uA[:], 1.0, None, OP.is_ge)
                else:
                    nc.gpsimd.tensor_scalar(sA, uA[:], 1.0, None, OP.is_ge)
                nbA = uvp.tile([128, NG, B_SH], F32, name=f"nbA{i}", tag="nb")
                if USE_STT:
                    nc.vector.scalar_tensor_tensor(
                        nbA[:], uA[:], 0.01, uA[:], OP.mult, OP.subtract)
                else:
                    aA = uvp.tile([128, NG, B_SH], F32, name=f"aA{i}", tag="a")
                    nc.vector.tensor_scalar_mul(aA[:], uA[:], 0.01)
                    nc.vector.tensor_tensor(nbA[:], aA[:], uA[:], OP.subtract)

                # step B (t = 2i+1)
                s1B = s1[:, :, 1, :]
                uB = uvp.tile([128, NG, B_SH], F32, name=f"uB{i}", tag="u")
                nc.vector.tensor_tensor(uB[:], s1B, nbA[:], OP.subtract)
                nc.vector.copy_predicated(uB[:], sA.bitcast(I32), s1B)
                sB = s_pair[:, :, 1, :]
                if S_ON_DVE:
                    nc.vector.tensor_scalar(sB, uB[:], 1.0, None, OP.is_ge)
                else:
                    nc.gpsimd.tensor_scalar(sB, uB[:], 1.0, None, OP.is_ge)
                nbB = uvp.tile([128, NG, B_SH], F32, name=f"nbB{i}", tag="nb")
                if USE_STT:
                    nc.vector.scalar_tensor_tensor(
                        nbB[:], uB[:], 0.01, uB[:], OP.mult, OP.subtract)
                else:
                    aB = uvp.tile([128, NG, B_SH], F32, name=f"aB{i}", tag="a")
                    nc.vector.tensor_scalar_mul(aB[:], uB[:], 0.01)
                    nc.vector.tensor_tensor(nbB[:], aB[:], uB[:], OP.subtract)

                nc.sync.dma_start(s_out[i], s_pair[:])

                s_pair_prev = s_pair
                nb_prev = nbB
                s_prev = sB

    nc.compile()
    return nc


def _prep_inputs(input_spikes, w_input_exc, w_inh_exc):
    x = np.ascontiguousarray(input_spikes, dtype=np.float32)     # (128, 784, 500)
    w_in = np.maximum(np.asarray(w_input_exc, dtype=np.float32), np.float32(0.0))
    mask = (np.float32(1.0) - np.eye(400, dtype=np.float32))
    w_ie = (np.maximum(np.asarray(w_inh_exc, dtype=np.float32),
                       np.float32(0.0)) * mask).astype(np.float32)

    w_in_pad = np.zeros((G0 * 128, 512), np.float32)
    w_in_pad[:784, :400] = w_in
    w_ie_pad = np.zeros((G1 * 128, 512), np.float32)
    w_ie_pad[:400, :400] = w_ie
    ident = np.eye(32, dtype=np.float32)

    in_maps = []
    for c in range(NCORES):
        xc = x[c * B_SH:(c + 1) * B_SH]                  # (16, 784, 500)
        xtc = np.zeros((G0 * 128, T, B_SH), np.float32)
        xtc[:784] = xc.transpose(1, 2, 0)                 # (784, 500, 16)
        in_maps.append({
            "xt": np.ascontiguousarray(xtc.reshape(G0, 128, T, B_SH)),
            "w_in": w_in_pad.reshape(G0, 128, 512),
            "w_ie": w_ie_pad.reshape(G1, 128, 512),
            "ident": ident,
        })
    return in_maps


def kernel(input_spikes, w_input_exc, w_inh_exc):
    from concourse.bass_utils import run_bass_kernel_spmd

    if "nc" not in _CACHE:
        _CACHE["nc"] = _build()
    nc = _CACHE["nc"]

    in_maps = _prep_inputs(input_spikes, w_input_exc, w_inh_exc)
    res = run_bass_kernel_spmd(nc, in_maps, core_ids=list(range(NCORES)))

    exc = np.empty((NCORES * B_SH, 400, T), np.float32)
    for c in range(NCORES):
        so = res.results[c]["s_out"]                      # (250, 128, 4, 2, 16)
        e = so.transpose(4, 2, 1, 0, 3).reshape(B_SH, NG * 128, T)
        exc[c * B_SH:(c + 1) * B_SH] = e[:, :400, :]
    inh = np.zeros_like(exc)
    inh[:, :, 1:] = exc[:, :, :-1]
    return exc, inh


# revision 28
# speedup vs baseline: 1.0496x; 1.0059x over previous
"""Bit-exact Trainium2 Bass kernel for the DiehlCook SNN reference.

Data-parallel over batch (128 -> 16 per core x 8 cores). Reproduces the
neuronxcc reference lowering bit-for-bit:
  - matmuls fp32 on PE, K tiled in 128-groups ascending, PSUM-accumulated in
    order (spikes are 0/1 so products are exact; operand swap verified bit-safe)
  - LIF elementwise as discrete IEEE f32 DVE/GPSIMD ops matching the HLO
    dataflow (sign-flipped decay carry nb = fl(u*0.01) - u is IEEE-exact)
  - inh_spikes == exc_spikes delayed one step (exact property), done on host.

Phase-1 (I_in[t] = x_t @ w_in, all t) is interleaved into the phase-2
recurrence loop so its PE/ACT work fills the recurrence's engine gaps.
"""
import numpy as np

T = 500
PAIRS = T // 2
B_SH = 16          # batch per core
NCORES = 8
G0 = 7             # k-groups for dot0 (784 -> 6*128 + 112)
G1 = 4             # k-groups for dot1 (400 padded to 512; pad rows stay 0)
NG = 4             # n-groups (400 -> 4*128; lanes >=400 stay identically 0)
CH_T = 16          # timesteps per phase-1 chunk (16*16 = 256 moving columns)
NCHUNK = (T + CH_T - 1) // CH_T
LOOKAHEAD = 1      # chunks emitted ahead of the pairs that consume them
INTERLEAVE = True  # merge phase-1 into the recurrence loop
USE_STT = True     # fused scalar_tensor_tensor decay (nb carry)
P1_COPY_ACT = False
S_ON_DVE = True    # is_ge + psum copy on DVE (fewer cross-engine hops)

_CACHE = {}


def _build():
    import concourse.bass as bass
    import concourse.tile as tile
    from concourse import bacc, mybir

    F32 = mybir.dt.float32
    I32 = mybir.dt.int32
    OP = mybir.AluOpType
    nchunk = NCHUNK
    pairs = PAIRS

    nc = bacc.Bacc("TRN2", target_bir_lowering=False, debug=False)

    xt = nc.dram_tensor("xt", [G0, 128, T, B_SH], F32, kind="ExternalInput")
    w_in = nc.dram_tensor("w_in", [G0, 128, 512], F32, kind="ExternalInput")
    w_ie = nc.dram_tensor("w_ie", [G1, 128, 512], F32, kind="ExternalInput")
    ident = nc.dram_tensor("ident", [32, 32], F32, kind="ExternalInput")
    s_out = nc.dram_tensor("s_out", [pairs, 128, NG, 2, B_SH], F32, kind="ExternalOutput")

    with tile.TileContext(nc) as tc:
        with (
            tc.tile_pool(name="const", bufs=1) as constp,
            tc.tile_pool(name="iinp", bufs=1) as iinp,
            tc.tile_pool(name="xtp", bufs=2) as xtp,
            tc.tile_pool(name="ps1", bufs=2, space="PSUM") as ps1,
            tc.tile_pool(name="bmps", bufs=2, space="PSUM") as bmps,
            tc.tile_pool(name="nmps", bufs=2, space="PSUM") as nmps,
            tc.tile_pool(name="bmsb", bufs=3) as bmsb,
            tc.tile_pool(name="s1p", bufs=4) as s1p,
            tc.tile_pool(name="sp", bufs=4) as sp,
            tc.tile_pool(name="uv", bufs=4) as uvp,
        ):
            wi_sb = constp.tile([128, G0, 512], F32)
            for g in range(G0):
                nc.sync.dma_start(wi_sb[:, g, :], w_in[g])
            wie_sb = constp.tile([128, G1, 512], F32)
            for g in range(G1):
                nc.sync.dma_start(wie_sb[:, g, :], w_ie[g])
            ident_sb = constp.tile([32, 32], F32)
            nc.sync.dma_start(ident_sb[:], ident[:])

            szero = constp.tile([128, NG, 2, B_SH], F32)
            nc.gpsimd.memset(szero[:], 0.0)
            nbzero = constp.tile([128, NG, B_SH], F32)
            nc.gpsimd.memset(nbzero[:], 0.0)

            iin_tiles = [None] * (pairs // 2)

            xts_by_ch = {}

            def emit_part(ch, ng):
                """dot0 for chunk ch, n-group ng; ng==0 also DMAs + allocs."""
                t0 = ch * CH_T
                t1 = min(T, t0 + CH_T)
                ccols = (t1 - t0) * B_SH
                npair = (t1 - t0) // 2
                if ng == 0:
                    xts = xtp.tile([128, G0, CH_T * B_SH], F32,
                                   name=f"xts{ch}", tag="xts")
                    xts_by_ch[ch] = xts
                    for g in range(G0):
                        kk = 128 if g < G0 - 1 else 112
                        nc.sync.dma_start(xts[0:kk, g, 0:ccols], xt[g, 0:kk, t0:t1, :])
                    for q in range(npair // 2):
                        iin_tiles[t0 // 4 + q] = iinp.tile(
                            [128, NG, 2, 2, B_SH], F32,
                            name=f"iin{t0 // 4 + q}", tag=f"iin{t0 // 4 + q}")
                xts = xts_by_ch[ch]
                ps = ps1.tile([128, 512], F32, name=f"ps{ch}_{ng}", tag="p1")
                for g in range(G0):
                    kk = 128 if g < G0 - 1 else 112
                    nc.tensor.matmul(
                        ps[:, 0:ccols],
                        wi_sb[0:kk, g, ng * 128:(ng + 1) * 128],
                        xts[0:kk, g, 0:ccols],
                        start=(g == 0),
                        stop=(g == G0 - 1),
                    )
                for q in range(npair // 2):
                    nc.vector.tensor_copy(
                        iin_tiles[t0 // 4 + q][:, ng],
                        ps[:, 64 * q:64 * q + 64].rearrange(
                            "n (p s b) -> n p s b", p=2, s=2, b=B_SH),
                    )

            PPC = CH_T // 2          # pairs covered per chunk
            if INTERLEAVE:
                for pi in range(NG):
                    emit_part(0, pi)
            else:
                for ch in range(nchunk):
                    for pi in range(NG):
                        emit_part(ch, pi)

            s_pair_prev = szero
            nb_prev = nbzero      # nb = fl(u*0.01) - u of prev step (spiked lanes unused)
            s_prev = szero[:, :, 0, :]

            for i in range(pairs):
                if INTERLEAVE and i % (PPC // NG) == 0:
                    pi = i // (PPC // NG) + NG
                    if pi // NG < nchunk:
                        emit_part(pi // NG, pi % NG)

                psb = bmps.tile([32, 400], F32, name=f"psb{i}", tag="bm")
                for g in range(G1):
                    nc.tensor.matmul(
                        psb[:],
                        s_pair_prev[:, g],
                        wie_sb[:, g, 0:400],
                        start=(g == 0),
                        stop=(g == G1 - 1),
                    )
                bm = bmsb.tile([32, 400], F32, name=f"bm{i}", tag="bmsb")
                if S_ON_DVE:
                    nc.vector.tensor_copy(bm[:], psb[:])
                else:
                    nc.scalar.copy(bm[:], psb[:])

                nm = nmps.tile([128, NG, 32], F32, name=f"nm{i}", tag="nm")
                nc.vector.memset(nm[:, 3, :], 0.0)
                for c in range(NG):
                    cw = 128 if c < 3 else 16
                    nc.tensor.transpose(
                        nm[0:cw, c, :],
                        bm[:, 128 * c:128 * c + cw],
                        ident_sb[:],
                    )

                s1 = s1p.tile([128, NG, 2, B_SH], F32, name=f"s1_{i}", tag="s1")
                nc.vector.tensor_tensor(
                    s1[:],
                    iin_tiles[i // 2][:, :, i % 2],
                    nm[:].rearrange("n c (s b) -> n c s b", s=2, b=B_SH),
                    OP.subtract,
                )

                s_pair = sp.tile([128, NG, 2, B_SH], F32, name=f"sp{i}", tag="spair")

                # step A (t = 2i): u = fl(s1 - nb_prev); spiked(prev) lanes -> s1
                s1A = s1[:, :, 0, :]
                uA = uvp.tile([128, NG, B_SH], F32, name=f"uA{i}", tag="u")
                nc.vector.tensor_tensor(uA[:], s1A, nb_prev[:], OP.subtract)
                nc.vector.copy_predicated(uA[:], s_prev.bitcast(I32), s1A)
                sA = s_pair[:, :, 0, :]
                if S_ON_DVE:
                    nc.vector.tensor_scalar(sA, uA[:], 1.0, None, OP.is_ge)
                else:
                    nc.gpsimd.tensor_scalar(sA, uA[:], 1.0, None, OP.is_ge)
                nbA = uvp.tile([128, NG, B_SH], F32, name=f"nbA{i}", tag="nb")
                if USE_STT:
                    nc.vector.scalar_tensor_tensor(
                        nbA[:], uA[:], 0.01, uA[:], OP.mult, OP.subtract)
                else:
                    aA = uvp.tile([128, NG, B_SH], F32, name=f"aA{i}", tag="a")
                    nc.vector.tensor_scalar_mul(aA[:], uA[:], 0.01)
                    nc.vector.tensor_tensor(nbA[:], aA[:], uA[:], OP.subtract)

                # step B (t = 2i+1)
                s1B = s1[:, :, 1, :]
                uB = uvp.tile([128, NG, B_SH], F32, name=f"uB{i}", tag="u")
                nc.vector.tensor_tensor(uB[:], s1B, nbA[:], OP.subtract)
                nc.vector.copy_predicated(uB[:], sA.bitcast(I32), s1B)
                sB = s_pair[:, :, 1, :]
                if S_ON_DVE:
                    nc.vector.tensor_scalar(sB, uB[:], 1.0, None, OP.is_ge)
                else:
                    nc.gpsimd.tensor_scalar(sB, uB[:], 1.0, None, OP.is_ge)
                nbB = uvp.tile([128, NG, B_SH], F32, name=f"nbB{i}", tag="nb")
                if USE_STT:
                    nc.vector.scalar_tensor_tensor(
                        nbB[:], uB[:], 0.01, uB[:], OP.mult, OP.subtract)
                else:
                    aB = uvp.tile([128, NG, B_SH], F32, name=f"aB{i}", tag="a")
                    nc.vector.tensor_scalar_mul(aB[:], uB[:], 0.01)
                    nc.vector.tensor_tensor(nbB[:], aB[:], uB[:], OP.subtract)

                nc.sync.dma_start(s_out[i], s_pair[:])

                s_pair_prev = s_pair
                nb_prev = nbB
                s_prev = sB

    nc.compile()
    return nc


def _prep_inputs(input_spikes, w_input_exc, w_inh_exc):
    x = np.ascontiguousarray(input_spikes, dtype=np.float32)     # (128, 784, 500)
    w_in = np.maximum(np.asarray(w_input_exc, dtype=np.float32), np.float32(0.0))
    mask = (np.float32(1.0) - np.eye(400, dtype=np.float32))
    w_ie = (np.maximum(np.asarray(w_inh_exc, dtype=np.float32),
                       np.float32(0.0)) * mask).astype(np.float32)

    w_in_pad = np.zeros((G0 * 128, 512), np.float32)
    w_in_pad[:784, :400] = w_in
    w_ie_pad = np.zeros((G1 * 128, 512), np.float32)
    w_ie_pad[:400, :400] = w_ie
    ident = np.eye(32, dtype=np.float32)

    in_maps = []
    for c in range(NCORES):
        xc = x[c * B_SH:(c + 1) * B_SH]                  # (16, 784, 500)
        xtc = np.zeros((G0 * 128, T, B_SH), np.float32)
        xtc[:784] = xc.transpose(1, 2, 0)                 # (784, 500, 16)
        in_maps.append({
            "xt": np.ascontiguousarray(xtc.reshape(G0, 128, T, B_SH)),
            "w_in": w_in_pad.reshape(G0, 128, 512),
            "w_ie": w_ie_pad.reshape(G1, 128, 512),
            "ident": ident,
        })
    return in_maps


def kernel(input_spikes, w_input_exc, w_inh_exc):
    from concourse.bass_utils import run_bass_kernel_spmd

    if "nc" not in _CACHE:
        _CACHE["nc"] = _build()
    nc = _CACHE["nc"]

    in_maps = _prep_inputs(input_spikes, w_input_exc, w_inh_exc)
    res = run_bass_kernel_spmd(nc, in_maps, core_ids=list(range(NCORES)))

    exc = np.empty((NCORES * B_SH, 400, T), np.float32)
    for c in range(NCORES):
        so = res.results[c]["s_out"]                      # (250, 128, 4, 2, 16)
        e = so.transpose(4, 2, 1, 0, 3).reshape(B_SH, NG * 128, T)
        exc[c * B_SH:(c + 1) * B_SH] = e[:, :400, :]
    inh = np.zeros_like(exc)
    inh[:, :, 1:] = exc[:, :, :-1]
    return exc, inh
